# revision 28
# baseline (speedup 1.0000x reference)
"""TRN2 Bass kernel: transformer Block (LN->MHA->2x residual->LN->MLP) for
B=32,N=512,C=768,H=12. Data-parallel over batch across 8 NeuronCores (4
items/core).

v5 design (v4 897us, v2 890us, baseline 1.11ms):
  - bf16 datapath (PSUM accumulation fp32). rel err ~7e-3 vs 2e-2 gate.
  - HAM-warmth: the attention inner loop alone is ScalarE(exp)-bound and
    leaves the PE idle in slivers, so HAM re-throttles the clock to 1.2GHz
    for ~68us per item (v4 trace). v5 interleaves the fc1 matmuls of the
    PREVIOUS item's tokens into the attention seams as dense PE filler:
    the PE stays busy (warm, 2.4GHz) and the MLP overlaps attention.
    fc1 output (+bias, pre-gelu) spills to DRAM as bf16 per j-tile;
    phase 2 reloads each 512-token chunk and runs ONE batched gelu on it.
  - qkv/proj weights: fp32 pieces on the sync HWDGE queue (line rate),
    PE transpose, cast evacs. fc1/fc2 casts (SWDGE) are gated behind a
    dummy read of wpT so they cannot steal early SDMA bandwidth.
  - attention: sub-head-interleaved scores/AV; deferred softmax normalize
    (DVE recip + 2.0-scaled broadcast matmul) one q4 group behind.
  - LN pipelining: LN1(b+1) at the q4=0 seam, LN2(b-1) at the q4=1 seam,
    transposes after proj(b); rstd via one Ln+Exp pair per 4-tile batch.
  - fc2 keeps x2 + h2T resident; LN2(3)+h2T(3)+fc1(3) run inside phase 2.
"""
import numpy as np
from contextlib import ExitStack

import concourse.bass as bass
import concourse.tile as tile
import concourse.bacc as bacc
from concourse import mybir
from concourse.bass_utils import run_bass_kernel_spmd
from concourse.masks import make_identity

F32 = mybir.dt.float32
F32R = mybir.dt.float32r
BF16 = mybir.dt.bfloat16
AF = mybir.ActivationFunctionType
ALU = mybir.AluOpType

B, N, C = 32, 512, 768
H, D = 12, 64
HID = 4 * C
EPS = 1e-5
NCORES = 8
BPC = B // NCORES            # batch items per core
T = BPC * N                  # tokens per core
CK = C // 128                # 6 contraction chunks over C
FQK = (2 * C) // 128         # 12 feature tiles for q+k
JH = HID // 128              # 24 hidden feature tiles
NT = N // 128                # 4 token tiles per item
NU = T // 128                # 16 token tiles per core
SCALE = D ** -0.5


def _bc(ap, p=128):
    """Broadcast a 1-D DRAM AP across p partitions (stride-0 partition dim)."""
    return bass.AP(tensor=ap.tensor, offset=ap.offset, ap=[[0, p]] + list(ap.ap))


def _emit(tc, io, ctx):
    nc = tc.nc

    consts = ctx.enter_context(tc.tile_pool(name="consts", bufs=1))
    small = ctx.enter_context(tc.tile_pool(name="small", bufs=4))
    x2p = ctx.enter_context(tc.tile_pool(name="x2p", bufs=1))
    dram = ctx.enter_context(tc.tile_pool(name="dram", bufs=1, space="DRAM"))
    psA = ctx.enter_context(tc.tile_pool(name="psA", bufs=4, space="PSUM"))
    psB = ctx.enter_context(tc.tile_pool(name="psB", bufs=2, space="PSUM"))

    # ---------------- constants ----------------
    ident32 = consts.tile([128, 128], F32)
    make_identity(nc, ident32)
    identb = consts.tile([128, 128], BF16)
    nc.vector.tensor_copy(out=identb, in_=ident32)
    epst = consts.tile([128, 1], F32)
    nc.vector.memset(epst, EPS)
    # head-pair broadcast matrices (value 2.0: residual doubling folded in)
    bcs = consts.tile([128, 128], F32)
    bca = consts.tile([128, 128], F32R)
    bcb = consts.tile([128, 128], F32R)
    nc.vector.memset(bcs, 0.0)
    nc.vector.memset(bcs[0:1, 0:64], 2.0)
    nc.vector.memset(bcs[32:33, 64:128], 2.0)
    nc.vector.tensor_copy(out=bca, in_=bcs)
    nc.vector.memset(bcs[0:1, 0:64], 0.0)
    nc.vector.memset(bcs[32:33, 64:128], 0.0)
    nc.vector.memset(bcs[64:65, 0:64], 2.0)
    nc.vector.memset(bcs[96:97, 64:128], 2.0)
    nc.vector.tensor_copy(out=bcb, in_=bcs)

    ones1f = consts.tile([1, 128], F32)
    nc.vector.memset(ones1f, 1.0)
    ones1r = consts.tile([1, 128], F32R)
    nc.vector.tensor_copy(out=ones1r, in_=ones1f)

    def bcast(vec_name, pool, tag=None, mul=1.0):
        """DMA a [C] vector into one partition, broadcast to [128, C] via a
        ones-row matmul (the stride-0 replicate DMA measured up to 90us)."""
        vf = small.tile([1, C], F32, tag="vf", bufs=1, name="vf")
        nc.scalar.dma_start(out=vf,
                            in_=io[vec_name].rearrange("(o c) -> o c", o=1))
        vr = small.tile([1, C], F32R, tag="vr", bufs=1, name="vr")
        nc.vector.tensor_copy(out=vr, in_=vf)
        bp = psB.tile([128, C], F32, tag="b", name="bp")
        for n0, nn in ((0, 512), (512, 256)):
            nc.tensor.matmul(bp[:, n0:n0 + nn], ones1r[:],
                             vr[:, n0:n0 + nn])
        if tag is None:
            out = pool.tile([128, C], F32, name="bc_" + vec_name)
        else:
            out = pool.tile([128, C], F32, tag=tag, bufs=1,
                            name="bc_" + vec_name)
        if mul != 1.0:
            nc.scalar.mul(out=out, in_=bp[:], mul=mul)
        else:
            nc.vector.tensor_copy(out=out, in_=bp[:])
        return out

    pb2_bc = bcast("proj_b", consts, mul=2.0)
    fc2b_bc = bcast("fc2_b", consts)

    # column-layout vectors via row-major load + PE transpose
    w1col = consts.tile([128, CK], F32)
    w2col = consts.tile([128, CK], F32)
    fc1b_t = consts.tile([128, JH], F32)
    with tc.tile_pool(name="colstage", bufs=2) as colst:
        for src, ncol, dst in ((io["ln1_w"], CK, w1col),
                               (io["ln2_w"], CK, w2col),
                               (io["fc1_b"], JH, fc1b_t)):
            rows = colst.tile([JH, 128], F32, tag="rows", name="rows")
            nc.scalar.dma_start(out=rows[0:ncol, :],
                                in_=src.rearrange("(k p) -> k p", p=128))
            cps = psA.tile([128, JH], F32, tag="a", name="cps")
            nc.tensor.transpose(cps[:, 0:ncol], rows[0:ncol, :],
                                ident32[0:ncol, 0:ncol])
            nc.vector.tensor_copy(out=dst[:, 0:ncol], in_=cps[:, 0:ncol])

    # b~ = ln_b / ln_w
    bt1 = consts.tile([128, C], BF16)
    bt2 = consts.tile([128, C], BF16)
    with tc.tile_pool(name="lnstage", bufs=1) as lnst:
        for wname, bname, dst in (("ln1_w", "ln1_b", bt1),
                                  ("ln2_w", "ln2_b", bt2)):
            wbc = bcast(wname, lnst, tag="wbc")
            bbc = bcast(bname, lnst, tag="bbc")
            winv = lnst.tile([128, C], F32, tag="winv", name="winv")
            nc.vector.reciprocal(out=winv, in_=wbc)
            nc.vector.tensor_mul(out=dst, in0=bbc, in1=winv)

    # x2 (attention residual, bf16) + h2T: resident across both phases
    x2 = x2p.tile([128, NU, C], BF16)
    h2T = x2p.tile([128, CK, T], BF16)
    # fc1 raw output (pre-gelu, bias added) spill for token chunks 0..2
    fraw = dram.tile([JH, 128, T], BF16)

    evac_ctr = [0]

    def evac(dst, src):
        if evac_ctr[0] % 2 == 0:
            nc.vector.tensor_copy(out=dst, in_=src)
        else:
            nc.scalar.copy(out=dst, in_=src)
        evac_ctr[0] += 1

    def transpose_to(h, dstT, tt, ident):
        for k in range(CK):
            tp = psA.tile([128, 128], h.dtype, tag="a", name="tp")
            nc.tensor.transpose(tp[:], h[:, k * 128:(k + 1) * 128], ident[:])
            evac(dstT[:, k, tt * 128:(tt + 1) * 128], tp[:])

    def ln_batch(x_tiles, bt, pool, tag):
        """Batched LN over 4 [128, C] tiles: one Ln+Exp pair for the whole
        group's rstd. Returns bf16 h tiles = (x-mu)*rstd + b/w."""
        n = len(x_tiles)
        mvs = small.tile([128, n, nc.vector.BN_AGGR_DIM], F32, tag="mvs",
                         name="mvs")
        for i, x_t in enumerate(x_tiles):
            st = small.tile([128, 3, nc.vector.BN_STATS_DIM], F32, tag="bnst",
                            name="st")
            for j in range(3):
                nc.vector.bn_stats(out=st[:, j, :],
                                   in_=x_t[:, 256 * j:256 * (j + 1)])
            nc.vector.bn_aggr(out=mvs[:, i, :], in_=st)
        lnv = small.tile([128, BPC], F32, tag="lnv", name="lnv")
        nc.scalar.activation(out=lnv[:, 0:n], in_=mvs[:, :, 1], func=AF.Ln,
                             bias=epst)
        rstds = small.tile([128, BPC], F32, tag="rstds", name="rstds")
        nc.scalar.activation(out=rstds[:, 0:n], in_=lnv[:, 0:n], func=AF.Exp,
                             scale=-0.5)
        hs = []
        for i, x_t in enumerate(x_tiles):
            ht = pool.tile([128, C], BF16, tag=tag, bufs=4, name="ht")
            nc.vector.tensor_scalar(out=ht, in0=x_t, scalar1=mvs[:, i, 0:1],
                                    scalar2=rstds[:, i:i + 1],
                                    op0=ALU.subtract, op1=ALU.mult)
            nc.vector.tensor_add(out=ht, in0=ht, in1=bt)
            hs.append(ht)
        return hs

    # ================= phase 1 =================
    with tc.tile_pool(name="wqkv", bufs=1) as wqp, \
         tc.tile_pool(name="wf1p", bufs=1) as wf1p, \
         tc.tile_pool(name="wstage", bufs=2) as wstage, \
         tc.tile_pool(name="p1", bufs=1) as p1, \
         tc.tile_pool(name="xio", bufs=4) as xio:
        wf1T = wf1p.tile([128, CK, HID], BF16)

        # qkv/proj: fp32 row-block pieces on the sync queue, PE transpose,
        # cast evacs to bf16
        wqkvT = wqp.tile([128, CK, 3 * C], BF16)
        wpT = wqp.tile([128, CK, C], BF16)
        for w_ap, nrows, dstT in ((io["qkv_w"], 3 * C, wqkvT),
                                  (io["proj_w"], C, wpT)):
            wr = w_ap.rearrange("(j p) c -> p j c", p=128)
            for j in range(nrows // 128):
                piece = wstage.tile([128, C], F32, tag="wstage", name="piece")
                nc.sync.dma_start(out=piece, in_=wr[:, j, :])
                for k in range(CK):
                    tp = psA.tile([128, 128], F32, tag="a", name="tp")
                    nc.tensor.transpose(tp[:], piece[:, k * 128:(k + 1) * 128],
                                        ident32[:])
                    evac(dstT[:, k, j * 128:(j + 1) * 128], tp[:])
        for k in range(CK):
            nc.vector.tensor_scalar(out=wqkvT[:, k, :], in0=wqkvT[:, k, :],
                                    scalar1=w1col[:, k:k + 1], scalar2=None,
                                    op0=ALU.mult)

        # fc weight DRAM bf16 images: SWDGE casts gated behind wpT so the
        # qkv/proj/x DMAs win the early SDMA bandwidth
        fc1_bf = dram.tile([HID, C], BF16)
        fc2_bf = dram.tile([C, HID], BF16)
        gate = small.tile([1, 8], BF16, tag="gate", name="gate")
        nc.gpsimd.tensor_copy(out=gate, in_=wpT[0:1, CK - 1, C - 8:C])
        nc.gpsimd.dma_start(out=fc1_bf, in_=io["fc1_w"])
        nc.gpsimd.dma_start(out=fc2_bf, in_=io["fc2_w"])

        def load_wf1(k):
            nc.sync.dma_start_transpose(out=wf1T[:, k, :],
                                        in_=fc1_bf[:, k * 128:(k + 1) * 128])
            nc.vector.tensor_scalar(out=wf1T[:, k, :], in0=wf1T[:, k, :],
                                    scalar1=w2col[:, k:k + 1], scalar2=None,
                                    op0=ALU.mult)

        def emit_x_loads(b):
            ts = []
            for tt in range(NT):
                x_t = xio.tile([128, C], F32, tag="xio", name="x_t")
                nc.scalar.dma_start(
                    out=x_t,
                    in_=io["x"][b * N + tt * 128:b * N + (tt + 1) * 128, :])
                ts.append(x_t)
            return ts

        def emit_qk(b, h0T):
            qk_sb = p1.tile([128, FQK, N], BF16, tag="qk", name="qk_sb")
            for j in range(FQK):
                qp = psA.tile([128, N], F32, tag="a", name="qp")
                for k in range(CK):
                    nc.tensor.matmul(qp[:], wqkvT[:, k, j * 128:(j + 1) * 128],
                                     h0T[:, k, :], start=(k == 0),
                                     stop=(k == CK - 1))
                evac(qk_sb[:, j, :], qp[:])
            return qk_sb

        def emit_v(b, h0T):
            v_sb = p1.tile([128, NT, H, 66], BF16, tag="v", name="v_sb")
            nc.vector.memset(v_sb[:, :, :, D:D + 1], 1.0)
            for tt in range(NT):
                vp = psB.tile([128, C], F32, tag="b", name="vp")
                for k in range(CK):
                    for n0, nn in ((0, 512), (512, 256)):
                        nc.tensor.matmul(vp[:, n0:n0 + nn],
                                         h0T[:, k, tt * 128:(tt + 1) * 128],
                                         wqkvT[:, k, 2 * C + n0:2 * C + n0 + nn],
                                         start=(k == 0), stop=(k == CK - 1))
                nc.vector.tensor_copy(out=v_sb[:, tt, :, 0:D],
                                      in_=vp.rearrange("p (h d) -> p h d", h=H))
            return v_sb

        def attn_q4(qk_sb, v_sb, q4, filler=None):
            """Scores+exp+AV for one q4 group (2 head pairs), sub-heads
            interleaved; one filler() call per c-step keeps the PE dense
            through the exp-gated stretches. Returns (rec4, orws)."""
            srow = p1.tile([128, N], F32, tag="srow", bufs=1, name="srow")
            nc.vector.memset(srow[0:97, :], 1.0)
            orws = []
            for pi in range(2):
                hp = 2 * q4 + pi
                orw = p1.tile([128, N], BF16, tag="orw", bufs=4, name="orw")
                avs = [psA.tile([D + 1, N], F32, tag="a", name="av")
                       for _ in range(2)]
                for c in range(NT):
                    exs = []
                    for sub in range(2):
                        p0 = 64 * sub
                        sc = psA.tile([128, N], F32, tag="a", name="sc")
                        nc.tensor.matmul(
                            sc[:],
                            qk_sb[p0:p0 + D, FQK // 2 + hp,
                                  c * 128:(c + 1) * 128],
                            qk_sb[p0:p0 + D, hp, :])
                        ex = p1.tile([128, N], BF16, tag="ex", bufs=2,
                                     name="ex")
                        nc.scalar.activation(out=ex, in_=sc[:], func=AF.Exp,
                                             scale=SCALE)
                        exs.append(ex)
                    # dense PE filler here covers the exp latency before the
                    # AV matmuls, keeping the PE continuously busy (HAM warm)
                    if filler is not None:
                        filler()
                    for sub in range(2):
                        h = 2 * hp + sub
                        nc.tensor.matmul(avs[sub][:], v_sb[:, c, h, 0:D + 1],
                                         exs[sub][:], start=(c == 0),
                                         stop=(c == NT - 1))
                for sub in range(2):
                    h = 2 * hp + sub
                    p0 = 64 * sub
                    r = 32 * (h % 4)
                    nc.vector.tensor_copy(out=srow[r:r + 1, :],
                                          in_=avs[sub][D:D + 1, :])
                    nc.scalar.copy(out=orw[p0:p0 + D, :], in_=avs[sub][0:D, :])
                orws.append(orw)
            rec4 = p1.tile([128, N], F32R, tag="rec", bufs=2, name="rec4")
            with nc.allow_low_precision(reason="softmax denom recip"):
                nc.vector.reciprocal(out=rec4[0:97, 0:N // 2],
                                     in_=srow[0:97, 0:N // 2])
                nc.vector.reciprocal(out=rec4[0:97, N // 2:N],
                                     in_=srow[0:97, N // 2:N])
            return rec4, orws

        def attn_norm(oT, q4, rec4, orws):
            for pi in range(2):
                hp = 2 * q4 + pi
                bcp = psA.tile([128, N], F32, tag="a", name="bcp")
                nc.tensor.matmul(bcp[:], (bca if pi == 0 else bcb)[0:97, :],
                                 rec4[0:97, :])
                nc.vector.tensor_mul(out=oT[:, hp, :], in0=bcp[:],
                                     in1=orws[pi])

        def emit_proj(b, oT):
            for tt in range(NT):
                pr = psB.tile([128, C], F32, tag="b", name="pr")
                for k in range(CK):
                    for n0, nn in ((0, 512), (512, 256)):
                        nc.tensor.matmul(pr[:, n0:n0 + nn],
                                         oT[:, k, tt * 128:(tt + 1) * 128],
                                         wpT[:, k, n0:n0 + nn],
                                         start=(k == 0), stop=(k == CK - 1))
                nc.vector.tensor_add(out=x2[:, b * NT + tt, :], in0=pr[:],
                                     in1=pb2_bc)

        def emit_fc1_j(q, j):
            """One fc1 j-tile for 512-token chunk q; bias added at evac;
            raw (pre-gelu) spill to DRAM bf16."""
            fp = psB.tile([128, 512], F32, tag="b", name="fp")
            for k in range(CK):
                nc.tensor.matmul(fp[:],
                                 wf1T[:, k, j * 128:(j + 1) * 128],
                                 h2T[:, k, q * 512:(q + 1) * 512],
                                 start=(k == 0), stop=(k == CK - 1))
            fr = p1.tile([128, 512], BF16, tag="fr", bufs=2, name="fr")
            if evac_ctr[0] % 2 == 0:
                nc.vector.tensor_scalar(out=fr, in0=fp[:],
                                        scalar1=fc1b_t[:, j:j + 1],
                                        scalar2=None, op0=ALU.add)
            else:
                nc.scalar.activation(out=fr, in_=fp[:], func=AF.Identity,
                                     bias=fc1b_t[:, j:j + 1])
            evac_ctr[0] += 1
            nc.scalar.dma_start(out=fraw[j, :, q * 512:(q + 1) * 512],
                                in_=fr)

        def mk_filler(q):
            state = [0]

            def f():
                if state[0] < JH:
                    emit_fc1_j(q, state[0])
                    state[0] += 1
            return f

        # ---- software-pipelined item loop ----
        xs = emit_x_loads(0)
        hs1 = {0: ln_batch(xs, bt1, p1, "lnt")}
        hs2 = {}
        h0T = p1.tile([128, CK, N], BF16, tag="hoT", bufs=2, name="h0T")
        for tt in range(NT):
            transpose_to(hs1[0][tt], h0T, tt, identb)
        for k in range(CK):
            load_wf1(k)

        for b in range(BPC):
            if b >= 1:
                # LN2(b-1) stats run on DVE while the PE does qk/v below
                hs2[b - 1] = ln_batch(
                    [x2[:, (b - 1) * NT + tt, :] for tt in range(NT)],
                    bt2, p1, "lnt")
            if b + 1 < BPC:
                xs_next = emit_x_loads(b + 1)
            qk_sb = emit_qk(b, h0T)
            v_sb = emit_v(b, h0T)
            filler = None
            if b >= 1:
                # h2T(b-1) transposes, then fc1(b-1) j-tiles interleaved one
                # per attention c-step (24 steps, 24 tiles): dense PE work
                # that keeps HAM warm through the exp-bound core
                for tt in range(NT):
                    transpose_to(hs2[b - 1][tt], h2T, (b - 1) * NT + tt,
                                 identb)
                filler = mk_filler(b - 1)
            oT = p1.tile([128, CK, N], BF16, tag="hoT", bufs=2, name="oT")
            res0 = attn_q4(qk_sb, v_sb, 0, filler=filler)
            res1 = attn_q4(qk_sb, v_sb, 1, filler=filler)
            if b + 1 < BPC:
                hs1[b + 1] = ln_batch(xs_next, bt1, p1, "lnt")
            attn_norm(oT, 0, *res0)
            res2 = attn_q4(qk_sb, v_sb, 2, filler=filler)
            attn_norm(oT, 1, *res1)
            if filler is not None:
                for _ in range(JH):
                    filler()
            attn_norm(oT, 2, *res2)
            emit_proj(b, oT)
            if b + 1 < BPC:
                for tt in range(NT):
                    transpose_to(hs1[b + 1][tt], h0T, tt, identb)

        # item 3's LN2 + h2T + fc1, still inside phase 1 (wf1T in scope)
        hs2[3] = ln_batch([x2[:, 3 * NT + tt, :] for tt in range(NT)],
                          bt2, p1, "lnt")
        for tt in range(NT):
            transpose_to(hs2[3][tt], h2T, 3 * NT + tt, identb)
        for j in range(JH):
            emit_fc1_j(3, j)

    # ================= phase 2: fc2 =================
    with tc.tile_pool(name="wf2p", bufs=1) as wf2p, \
         tc.tile_pool(name="p2", bufs=1) as p2:
        wf2T = wf2p.tile([128, JH, C], BF16)
        for k in range(JH):
            nc.sync.dma_start_transpose(out=wf2T[:, k, :],
                                        in_=fc2_bf[:, k * 128:(k + 1) * 128])

        def emit_fc2(q, gq):
            for tt in range(4):
                u = q * 4 + tt
                x2pb = p2.tile([128, C], F32, tag="x2pb", bufs=2, name="x2pb")
                nc.gpsimd.tensor_add(out=x2pb, in0=x2[:, u, :], in1=fc2b_bc)
                f2 = psB.tile([128, C], F32, tag="b", name="f2")
                for kk in range(JH):
                    for n0, nn in ((0, 512), (512, 256)):
                        nc.tensor.matmul(f2[:, n0:n0 + nn],
                                         gq[:, kk, tt * 128:(tt + 1) * 128],
                                         wf2T[:, kk, n0:n0 + nn],
                                         start=(kk == 0), stop=(kk == JH - 1))
                o_t = p2.tile([128, C], F32, tag="ot", bufs=2, name="o_t")
                nc.vector.tensor_add(out=o_t, in0=f2[:], in1=x2pb)
                nc.sync.dma_start(
                    out=io["out"][u * 128:(u + 1) * 128, :], in_=o_t)

        # reload the raw fc1 spill per chunk; one batched in-place gelu each
        def load_g(q):
            gq = p2.tile([128, JH, 512], BF16, tag="g", bufs=3, name="gq")
            nc.sync.dma_start(
                out=gq,
                in_=fraw[:, :, q * 512:(q + 1) * 512].rearrange(
                    "j p t -> p j t"))
            nc.scalar.activation(out=gq, in_=gq, func=AF.Gelu)
            return gq

        gs = [load_g(0), load_g(1)]
        for q in range(4):
            if q + 2 < 4:
                gs.append(load_g(q + 2))
            emit_fc2(q, gs[q])


_CACHE = {}


def _build():
    if "nc" in _CACHE:
        return _CACHE["nc"]
    nc = bacc.Bacc("TRN2", target_bir_lowering=False, debug=False,
                   num_devices=NCORES)
    io = {}
    io["x"] = nc.dram_tensor("x", [T, C], F32, kind="ExternalInput").ap()
    for name, shape in [("ln1_w", [C]), ("ln1_b", [C]), ("qkv_w", [3 * C, C]),
                        ("proj_w", [C, C]), ("proj_b", [C]), ("ln2_w", [C]),
                        ("ln2_b", [C]), ("fc1_w", [HID, C]), ("fc1_b", [HID]),
                        ("fc2_w", [C, HID]), ("fc2_b", [C])]:
        io[name] = nc.dram_tensor(name, shape, F32, kind="ExternalInput").ap()
    io["out"] = nc.dram_tensor("out", [T, C], F32, kind="ExternalOutput").ap()

    with tile.TileContext(nc) as tc:
        with ExitStack() as ctx:
            _emit(tc, io, ctx)
    nc.compile()
    _CACHE["nc"] = nc
    return nc


def kernel(**inputs):
    nc = _build()
    arrs = {k: np.ascontiguousarray(np.asarray(v, dtype=np.float32))
            for k, v in inputs.items()}
    x = arrs.pop("x").reshape(B, N, C)
    in_maps = []
    for c in range(NCORES):
        m = dict(arrs)
        m["x"] = np.ascontiguousarray(x[c * BPC:(c + 1) * BPC].reshape(T, C))
        in_maps.append(m)
    res = run_bass_kernel_spmd(nc, in_maps, core_ids=list(range(NCORES)))
    out = np.concatenate(
        [r["out"].reshape(BPC, N, C) for r in res.results], axis=0)
    return out.astype(np.float32)


if __name__ == "__main__":
    rng = np.random.default_rng(0)
    ins = {
        "x": rng.standard_normal((B, N, C), dtype=np.float32),
        "ln1_w": np.ones(C, np.float32), "ln1_b": np.zeros(C, np.float32),
        "qkv_w": rng.standard_normal((3 * C, C), dtype=np.float32) / np.sqrt(C),
        "proj_w": rng.standard_normal((C, C), dtype=np.float32) / np.sqrt(C),
        "proj_b": np.zeros(C, np.float32),
        "ln2_w": np.ones(C, np.float32), "ln2_b": np.zeros(C, np.float32),
        "fc1_w": rng.standard_normal((HID, C), dtype=np.float32) / np.sqrt(C),
        "fc1_b": np.zeros(HID, np.float32),
        "fc2_w": rng.standard_normal((C, HID), dtype=np.float32) / np.sqrt(HID),
        "fc2_b": np.zeros(C, np.float32),
    }
    out = kernel(**ins)
    print("out", out.shape, out.dtype, np.abs(out).max())
